# revision 8
# baseline (speedup 1.0000x reference)
"""Trainium2 Bass kernel for nn_MemoryAugmentedModel (gnn_message_passing).

Math: the reference only consumes row N-1 of the GAT output, so the dense
[N,N,H] attention collapses:
  out[-1] = (1/H) * sum_h gat_w_h @ (softmax_j(lrelu(a_dst[-1,h]+a_src[j,h])) @ nf) + gat_bias
with a_src = nf @ V_src^T, V_src[h] = att_src[h] @ gat_w_h  (same for dst).
Then LayerNorm -> proj/LoRA offset -> embedding gather with offset added to
each sequence's first token.

Sharding (8 cores): gat_w / node_features split by input-feature columns
(e-chunks of 256) -> partial attention logits (AllReduce #1) -> replicated
softmax -> per-core agg over its e-chunk -> partial out[-1] (AllReduce #2)
-> replicated LayerNorm -> proj/LoRA sharded by output rows (AllGather #3)
-> each core gathers 1024 of the 8192 output embedding rows (table
replicated); a per-core mask input applies the first-token offset add.
A tiny warmup AllReduce at t=0 absorbs the ~70us first-collective init.
"""

import os
import sys
import types

import numpy as np

NCORES = 8
N = 2048
D = 2048
H = 4
R = 32
V = 32000
B = 4
S = 2048

EC = D // NCORES          # 256: e-columns (input features) per core
FC = D // NCORES          # 256: offset rows per core
ROWS = (B * S) // NCORES  # 1024: output embedding rows per core
NG = ROWS // 128          # 8 gather groups per core
NU = D // 128             # 16: 128-row chunks of a length-D axis
NT = (H * D) // 128       # 64: 128-row strips of gat_w

_CACHE = {}


def _install_ntff_shim():
    """Register the axon NTFF profile hook missing from this image's antenv."""
    if "antenv.axon_hooks" in sys.modules:
        return
    try:
        import antenv
        from trn_agent_boot.trn_boot import _ntff_profile_via_ctypes
    except Exception:
        return
    mod = types.ModuleType("antenv.axon_hooks")
    mod._hook = None
    mod.set_axon_ntff_profile_hook = lambda h: setattr(mod, "_hook", h)
    mod.get_axon_ntff_profile_hook = lambda: mod._hook
    sys.modules["antenv.axon_hooks"] = mod
    antenv.axon_hooks = mod
    try:
        mod.set_axon_ntff_profile_hook(
            _ntff_profile_via_ctypes("/opt/axon/libaxon_pjrt.so")
        )
    except Exception:
        pass


def _build():
    import concourse.bacc as bacc
    import concourse.bass as bass
    import concourse.tile as tile
    from concourse import mybir
    from concourse.masks import make_identity

    f32 = mybir.dt.float32
    bf16 = mybir.dt.bfloat16
    i32 = mybir.dt.int32
    RG = [list(range(NCORES))]

    nc = bacc.Bacc("TRN2", target_bir_lowering=False, debug=False,
                   num_devices=NCORES)

    din = lambda name, shape, dt: nc.dram_tensor(name, shape, dt, kind="ExternalInput").ap()
    w_nat = din("w_nat", [NT, 128, EC], bf16)
    w_tr = din("w_tr", [2 * 128, H * D], bf16)
    nf_pre = din("nf_pre", [128, NU, EC], bf16)
    nf_tr = din("nf_tr", [2 * 128, N], bf16)
    att_pre = din("att_pre", [128, NU, 2 * H], bf16)
    proj_pre = din("proj_pre", [128, NU, FC], bf16)
    projb_r = din("projb_r", [1, FC], f32)
    lora_a_pre = din("lora_a_pre", [128, NU, R], bf16)
    lora_bt = din("lora_bt", [R, FC], bf16)
    gbias_r = din("gbias_r", [128, NU], f32)
    gamma_r = din("gamma_r", [128, NU], f32)
    beta_r = din("beta_r", [128, NU], f32)
    ids_r = din("ids_r", [128, NG], i32)
    sel_c = din("sel_c", [R, 2], f32)
    embed = din("embed", [V, D], f32)
    mask = din("mask", [1, 1], f32)

    out_sl = nc.dram_tensor("out_sl", [ROWS, D], f32, kind="ExternalOutput").ap()

    with tile.TileContext(nc) as tc:
        import contextlib
        ctx = contextlib.ExitStack()
        with ctx:
            const = ctx.enter_context(tc.tile_pool(name="const", bufs=1))
            strips = ctx.enter_context(tc.tile_pool(name="strips", bufs=32))
            embp = ctx.enter_context(tc.tile_pool(name="embp", bufs=NG))
            dram = ctx.enter_context(tc.tile_pool(name="dram", bufs=1, space="DRAM"))

            # ---------- embedding gather (independent; runs all kernel long) ----
            ids_sb = const.tile([128, NG], i32)
            nc.sync.dma_start(ids_sb[:], ids_r[:])
            emb_tiles = []
            for g in range(NG):
                et = embp.tile([128, D], f32, name=f"emb{g}", tag="emb")
                nc.gpsimd.indirect_dma_start(
                    out=et[:], out_offset=None, in_=embed[:, :],
                    in_offset=bass.IndirectOffsetOnAxis(ap=ids_sb[:, g:g + 1], axis=0),
                )
                emb_tiles.append(et)
                if g > 0:
                    nc.scalar.dma_start(out_sl[g * 128:(g + 1) * 128, :], et[:])

            # ---------- warmup collective (absorbs first-collective init) ------
            wu_sb = const.tile([128, 1], f32)
            nc.vector.memset(wu_sb[:], 0.0)
            wu_in = dram.tile([128, 1], f32, tag="wu_in")
            wu_out = dram.tile([128, 1], f32, tag="wu_out")
            nc.gpsimd.dma_start(wu_in[:], wu_sb[:])
            nc.gpsimd.collective_compute(
                "AllReduce", mybir.AluOpType.add, replica_groups=RG,
                ins=[wu_in[:].opt()], outs=[wu_out[:].opt()])
            wu_back = const.tile([128, 1], f32)
            nc.gpsimd.dma_start(wu_back[:], wu_out[:])

            # ---------- constants ---------------------------------------------
            att_sb = const.tile([128, NU, 2 * H], bf16)
            nc.sync.dma_start(att_sb[:], att_pre[:])
            wt_sb = []
            for half in range(2):
                t = const.tile([128, H * D], bf16, tag=f"wt{half}")
                nc.sync.dma_start(t[:], w_tr[half * 128:(half + 1) * 128, :])
                wt_sb.append(t)
            nft_sb = []
            for half in range(2):
                t = const.tile([128, N], bf16, tag=f"nft{half}")
                nc.sync.dma_start(t[:], nf_tr[half * 128:(half + 1) * 128, :])
                nft_sb.append(t)
            nf_sb = const.tile([128, NU, EC + 1], bf16)
            nc.sync.dma_start(nf_sb[:, :, 0:EC], nf_pre[:])
            nc.vector.memset(nf_sb[:, :, EC:EC + 1], 1.0)
            proj_sb = const.tile([128, NU, FC], bf16)
            nc.sync.dma_start(proj_sb[:], proj_pre[:])
            projb_sb = const.tile([1, FC], f32)
            nc.sync.dma_start(projb_sb[:], projb_r[:])
            lat_sb = const.tile([128, NU, R], bf16)
            nc.sync.dma_start(lat_sb[:], lora_a_pre[:])
            lbt_sb = const.tile([R, FC], bf16)
            nc.sync.dma_start(lbt_sb[:], lora_bt[:])
            gbias_sb = const.tile([128, NU], f32)
            nc.sync.dma_start(gbias_sb[:], gbias_r[:])
            gamma_sb = const.tile([128, NU], f32)
            nc.sync.dma_start(gamma_sb[:], gamma_r[:])
            beta_sb = const.tile([128, NU], f32)
            nc.sync.dma_start(beta_sb[:], beta_r[:])
            mask_sb = const.tile([1, 1], f32)
            nc.sync.dma_start(mask_sb[:], mask[:])
            ident_sb = const.tile([128, 128], bf16)
            make_identity(nc, ident_sb[:])
            ones1_sb = const.tile([1, 128], f32)
            nc.vector.memset(ones1_sb[:], 1.0)
            ones128_sb = const.tile([128, 1], f32)
            nc.vector.memset(ones128_sb[:], 1.0)
            sel_sb = const.tile([R, 2], f32)
            nc.sync.dma_start(sel_sb[:], sel_c[:])

            # ---------- phase 1: V = att @ W, partial a = nf @ V^T -------------
            v_sb = [const.tile([128, 2 * H], bf16, name=f"vsb{i}", tag=f"v{i}") for i in range(2)]
            a_loc = const.tile([128, 128], f32)
            with tc.tile_pool(name="ppv", bufs=1, space="PSUM") as ppv, \
                 tc.tile_pool(name="ppa", bufs=1, space="PSUM") as ppa:
                ps_v = ppv.tile([128, 2, 2, H], f32)  # [e_inner, e_half, src/dst, h]
                strip_tiles = {}
                for h in range(H):
                    for half in range(2):
                        for u in range(NU):
                            st_idx = h * NU + u
                            if half == 0:
                                st = strips.tile([128, EC], bf16, tag="wstrip")
                                nc.sync.dma_start(st[:], w_nat[st_idx, :, :])
                                strip_tiles[st_idx] = st
                            st = strip_tiles[st_idx]
                            nc.tensor.matmul(
                                out=ps_v[:, half, :, h],
                                lhsT=st[:, half * 128:(half + 1) * 128],
                                rhs=att_sb[:, u, :].rearrange("p (s h) -> p s h", s=2)[:, :, h],
                                start=(u == 0), stop=(u == NU - 1),
                            )
                for half in range(2):
                    nc.vector.tensor_copy(out=v_sb[half][:], in_=ps_v[:, half, :, :])

                ps_a = ppa.tile([128, 128], f32)  # [j_inner, j_outer*8 + col]
                for jc in range(NU):
                    for half in range(2):
                        nc.tensor.matmul(
                            out=ps_a[:, jc * 8:(jc + 1) * 8],
                            lhsT=nft_sb[half][:, jc * 128:(jc + 1) * 128],
                            rhs=v_sb[half][:],
                            start=(half == 0), stop=(half == 1),
                        )
                nc.vector.tensor_copy(out=a_loc[:], in_=ps_a[:])

            # ---------- AllReduce #1: logits ----------------------------------
            ar1_in = dram.tile([128, 128], f32, tag="ar1i")
            ar1_out = dram.tile([128, 128], f32, tag="ar1o")
            nc.gpsimd.dma_start(ar1_in[:], a_loc[:])
            nc.gpsimd.collective_compute(
                "AllReduce", mybir.AluOpType.add, replica_groups=RG,
                ins=[ar1_in[:].opt()], outs=[ar1_out[:].opt()])
            a_sb = const.tile([128, 128], f32)
            nc.gpsimd.dma_start(a_sb[:], ar1_out[:])
            # a_dst for target node N-1: j=2047 -> j_outer=15, p=127, cols 4..7
            dst1_sb = const.tile([1, H], f32)
            nc.gpsimd.dma_start(dst1_sb[:], ar1_out[127:128, 15 * 8 + 4:15 * 8 + 8])

            # ---------- softmax weights (replicated) --------------------------
            wu_exp = const.tile([128, NU, H], bf16)
            with tc.tile_pool(name="ppd", bufs=1, space="PSUM") as ppd:
                ps_dst = ppd.tile([128, H], f32)
                nc.tensor.matmul(out=ps_dst[:], lhsT=ones1_sb[:], rhs=dst1_sb[:],
                                 start=True, stop=True)
                dstb_sb = const.tile([128, H], f32)
                nc.vector.tensor_copy(out=dstb_sb[:], in_=ps_dst[:])
            a_src = a_sb[:].rearrange("p (u c) -> p u c", c=8)[:, :, 0:H]
            dstb_b = bass.AP(tensor=dstb_sb[:].tensor, offset=dstb_sb[:].offset,
                             ap=[dstb_sb[:].ap[0], [0, NU], [1, H]])
            l_sb = const.tile([128, NU, H], f32)
            nc.vector.tensor_tensor(out=l_sb[:], in0=a_src, in1=dstb_b,
                                    op=mybir.AluOpType.add)
            l2_sb = const.tile([128, NU, H], f32)
            nc.vector.tensor_scalar_mul(l2_sb[:], l_sb[:], 0.2)
            nc.vector.tensor_tensor(out=l_sb[:], in0=l_sb[:], in1=l2_sb[:],
                                    op=mybir.AluOpType.max)
            nc.scalar.activation(out=wu_exp[:], in_=l_sb[:],
                                 func=mybir.ActivationFunctionType.Exp)

            # ---------- agg = attnU^T @ [nf | 1] ; normalize ------------------
            aggT_sb = [const.tile([128, H], bf16, name=f"aggT{i}", tag=f"aggT{i}") for i in range(2)]
            with tc.tile_pool(name="ppg", bufs=1, space="PSUM") as ppg, \
                 tc.tile_pool(name="ppgt", bufs=2, space="PSUM") as ppgt:
                ps_agg = ppg.tile([H, EC + 1], f32)
                for u in range(NU):
                    nc.tensor.matmul(
                        out=ps_agg[:], lhsT=wu_exp[:, u, :], rhs=nf_sb[:, u, :],
                        start=(u == 0), stop=(u == NU - 1))
                rz_sb = const.tile([H, 1], f32)
                nc.vector.reciprocal(out=rz_sb[:], in_=ps_agg[:, EC:EC + 1])
                nc.scalar.mul(rz_sb[:], rz_sb[:], 1.0 / H)
                aggn_sb = const.tile([H, EC], bf16)
                nc.vector.tensor_scalar_mul(aggn_sb[:], ps_agg[:, 0:EC], rz_sb[:])
                for half in range(2):
                    ps_t = ppgt.tile([128, H], bf16, tag="pst")
                    nc.tensor.transpose(out=ps_t[:],
                                        in_=aggn_sb[:, half * 128:(half + 1) * 128],
                                        identity=ident_sb[0:H, 0:H])
                    nc.vector.tensor_copy(out=aggT_sb[half][:], in_=ps_t[:])

            # ---------- partial out[-1] ---------------------------------------
            ar2_in = dram.tile([128, NU], f32, tag="ar2i")
            ar2_out = dram.tile([128, NU], f32, tag="ar2o")
            with tc.tile_pool(name="ppo", bufs=1, space="PSUM") as ppo:
                ps_out = ppo.tile([128, NU], f32)
                for do in range(NU):
                    for h in range(H):
                        for half in range(2):
                            q0 = (h * NU + do) * 128
                            nc.tensor.matmul(
                                out=ps_out[:, do:do + 1],
                                lhsT=wt_sb[half][:, q0:q0 + 128],
                                rhs=aggT_sb[half][:, h:h + 1],
                                start=(h == 0 and half == 0),
                                stop=(h == H - 1 and half == 1),
                            )
                outp_sb = const.tile([128, NU], f32)
                nc.vector.tensor_copy(out=outp_sb[:], in_=ps_out[:])
            nc.gpsimd.dma_start(ar2_in[:], outp_sb[:])
            nc.gpsimd.collective_compute(
                "AllReduce", mybir.AluOpType.add, replica_groups=RG,
                ins=[ar2_in[:].opt()], outs=[ar2_out[:].opt()])

            # ---------- LayerNorm (replicated) --------------------------------
            lnst_sb = const.tile([128, 2 * NU], f32)
            nc.gpsimd.dma_start(lnst_sb[:, 0:NU], ar2_out[:])
            nc.vector.tensor_add(lnst_sb[:, 0:NU], lnst_sb[:, 0:NU], gbias_sb[:])
            nc.scalar.square(out=lnst_sb[:, NU:2 * NU], in_=lnst_sb[:, 0:NU])
            mem_sb = const.tile([128, NU], f32)
            memb_sb = const.tile([128, NU], bf16)
            with tc.tile_pool(name="ppl", bufs=1, space="PSUM") as ppl:
                ps_l1 = ppl.tile([2 * NU, 1], f32, tag="l1")
                nc.tensor.matmul(out=ps_l1[:], lhsT=lnst_sb[:], rhs=ones128_sb[:],
                                 start=True, stop=True)
                l1_sb = const.tile([2 * NU, 1], f32)
                nc.vector.tensor_copy(out=l1_sb[:], in_=ps_l1[:])
                l1b = bass.AP(tensor=l1_sb[:].tensor, offset=l1_sb[:].offset,
                              ap=[l1_sb[:].ap[0], [0, 2]])
                l1m_sb = const.tile([2 * NU, 2], f32)
                nc.vector.tensor_tensor(out=l1m_sb[:], in0=sel_sb[:], in1=l1b,
                                        op=mybir.AluOpType.mult)
                ps_l2 = ppl.tile([1, 2], f32, tag="l2")
                nc.tensor.matmul(out=ps_l2[:], lhsT=ones128_sb[0:32, :],
                                 rhs=l1m_sb[:], start=True, stop=True)
                s12_sb = const.tile([1, 2], f32)
                nc.vector.tensor_copy(out=s12_sb[:], in_=ps_l2[:])
                ps_b = ppl.tile([128, 2], f32, tag="bc")
                nc.tensor.matmul(out=ps_b[:], lhsT=ones1_sb[:], rhs=s12_sb[:],
                                 start=True, stop=True)
                bc_sb = const.tile([128, 2], f32)
                nc.vector.tensor_copy(out=bc_sb[:], in_=ps_b[:])
            mu_sb = const.tile([128, 1], f32)
            nc.vector.tensor_scalar_mul(mu_sb[:], bc_sb[:, 0:1], 1.0 / D)
            m2_sb = const.tile([128, 1], f32)
            nc.vector.tensor_scalar_mul(m2_sb[:], bc_sb[:, 1:2], 1.0 / D)
            var_sb = const.tile([128, 1], f32)
            nc.vector.tensor_tensor(out=var_sb[:], in0=mu_sb[:], in1=mu_sb[:],
                                    op=mybir.AluOpType.mult)
            nc.vector.tensor_tensor(out=var_sb[:], in0=m2_sb[:], in1=var_sb[:],
                                    op=mybir.AluOpType.subtract)
            eps_sb = const.tile([128, 1], f32)
            nc.vector.memset(eps_sb[:], 1e-5)
            rstd_sb = const.tile([128, 1], f32)
            nc.scalar.activation(out=rstd_sb[:], in_=var_sb[:],
                                 func=mybir.ActivationFunctionType.Sqrt,
                                 bias=eps_sb[:], scale=1.0)
            nc.vector.reciprocal(out=rstd_sb[:], in_=rstd_sb[:])
            nc.vector.tensor_scalar(out=mem_sb[:], in0=lnst_sb[:, 0:NU],
                                    scalar1=mu_sb[:], scalar2=rstd_sb[:],
                                    op0=mybir.AluOpType.subtract,
                                    op1=mybir.AluOpType.mult)
            nc.vector.tensor_mul(mem_sb[:], mem_sb[:], gamma_sb[:])
            nc.vector.tensor_add(mem_sb[:], mem_sb[:], beta_sb[:])
            # consume warmup output (zeros) so DCE cannot drop the warmup AR
            nc.vector.tensor_add(mem_sb[:, 0:1], mem_sb[:, 0:1], wu_back[:])
            nc.vector.tensor_copy(out=memb_sb[:], in_=mem_sb[:])

            # ---------- proj + LoRA offset chunk ------------------------------
            ag3_in = dram.tile([1, FC], f32, tag="ag3i")
            ag3_out = dram.tile([NCORES, FC], f32, tag="ag3o")
            with tc.tile_pool(name="ppp", bufs=1, space="PSUM") as ppp:
                ps_lt = ppp.tile([R, 1], f32, tag="lt")
                for u in range(NU):
                    nc.tensor.matmul(out=ps_lt[:], lhsT=lat_sb[:, u, :],
                                     rhs=memb_sb[:, u:u + 1],
                                     start=(u == 0), stop=(u == NU - 1))
                lt2_sb = const.tile([R, 1], bf16)
                nc.scalar.mul(lt2_sb[:], ps_lt[:], 2.0)  # LORA_SCALE = 64/32
                ps_pj = ppp.tile([1, FC], f32, tag="pj")
                for u in range(NU):
                    nc.tensor.matmul(
                        out=ps_pj[:], lhsT=memb_sb[:, u:u + 1],
                        rhs=proj_sb[:, u, :], start=(u == 0), stop=False)
                nc.tensor.matmul(out=ps_pj[:], lhsT=lt2_sb[:], rhs=lbt_sb[:],
                                 start=False, stop=True)
                off_sb = const.tile([1, FC], f32)
                nc.vector.tensor_add(off_sb[:], ps_pj[:], projb_sb[:])
            nc.gpsimd.dma_start(ag3_in[:], off_sb[:])
            nc.gpsimd.collective_compute(
                "AllGather", mybir.AluOpType.bypass, replica_groups=RG,
                ins=[ag3_in[:].opt()], outs=[ag3_out[:].opt()])

            # ---------- first-token offset add + final row group --------------
            # ag3_out[r, f] = offset[r*256 + f] -> contiguous [1, 2048] row
            off_row = const.tile([1, D], f32)
            nc.gpsimd.dma_start(off_row[:], ag3_out[:].rearrange("r f -> (r f)")[None, :])
            or2 = off_row[:]
            nc.vector.tensor_scalar_mul(or2, or2, mask_sb[:])
            nc.vector.tensor_add(emb_tiles[0][0:1, :], emb_tiles[0][0:1, :], or2)
            nc.scalar.dma_start(out_sl[0:128, :], emb_tiles[0][:])

    nc.compile()
    return nc


def _prep_inputs(inputs):
    import ml_dtypes
    bf16 = ml_dtypes.bfloat16

    nf = np.asarray(inputs["node_features"], dtype=np.float32)
    ids = np.asarray(inputs["input_ids"], dtype=np.int32).reshape(-1)
    gw = np.asarray(inputs["gat_w"], dtype=np.float32)
    att_src = np.asarray(inputs["att_src"], dtype=np.float32)
    att_dst = np.asarray(inputs["att_dst"], dtype=np.float32)
    gbias = np.asarray(inputs["gat_bias"], dtype=np.float32)
    gamma = np.asarray(inputs["ln_gamma"], dtype=np.float32)
    beta = np.asarray(inputs["ln_beta"], dtype=np.float32)
    pw = np.asarray(inputs["proj_w"], dtype=np.float32)
    pb = np.asarray(inputs["proj_b"], dtype=np.float32)
    la = np.asarray(inputs["lora_a"], dtype=np.float32)
    lb = np.asarray(inputs["lora_b"], dtype=np.float32)
    emb = np.ascontiguousarray(np.asarray(inputs["embed"], dtype=np.float32))

    def chunked(vec, parts=2):  # [parts*128] -> [128, parts]
        return np.ascontiguousarray(vec.reshape(parts, 128).T)

    def pre3(m, inner):  # [NU*128, inner] -> [128, NU, inner]
        return np.ascontiguousarray(
            m.reshape(NU, 128, inner).transpose(1, 0, 2).astype(bf16))

    att_t = np.concatenate([att_src.T, att_dst.T], axis=1)  # [D, 2H]
    att_pre = pre3(att_t, 2 * H)
    lora_a_pre = pre3(la.T, R)
    gbias_r = chunked(gbias, NU)
    gamma_r = chunked(gamma, NU)
    beta_r = chunked(beta, NU)

    sel_mat = np.zeros((R, 2), dtype=np.float32)
    sel_mat[0:16, 0] = 1.0
    sel_mat[16:32, 1] = 1.0

    in_maps = []
    for c in range(NCORES):
        ech = slice(c * EC, (c + 1) * EC)
        fch = slice(c * FC, (c + 1) * FC)
        w_sl = gw[:, ech]
        nf_sl = nf[:, ech]
        m = {
            "w_nat": np.ascontiguousarray(w_sl.reshape(NT, 128, EC).astype(bf16)),
            "w_tr": np.ascontiguousarray(w_sl.T.astype(bf16)),
            "nf_pre": pre3(nf_sl, EC),
            "nf_tr": np.ascontiguousarray(nf_sl.T.astype(bf16)),
            "att_pre": att_pre,
            "proj_pre": pre3(pw[fch, :].T, FC),
            "projb_r": np.ascontiguousarray(pb[fch].reshape(1, FC)),
            "lora_a_pre": lora_a_pre,
            "lora_bt": np.ascontiguousarray(lb[fch, :].T.astype(bf16)),
            "gbias_r": gbias_r,
            "gamma_r": gamma_r,
            "beta_r": beta_r,
            "ids_r": np.ascontiguousarray(
                ids[c * ROWS:(c + 1) * ROWS].reshape(NG, 128).T),
            "embed": emb,
            "sel_c": sel_mat,
            "mask": np.array([[1.0 if c % 2 == 0 else 0.0]], dtype=np.float32),
        }
        in_maps.append(m)
    return in_maps


def kernel(**inputs):
    _install_ntff_shim()
    from concourse.bass_utils import run_bass_kernel_spmd

    if "nc" not in _CACHE:
        _CACHE["nc"] = _build()
    nc = _CACHE["nc"]

    in_maps = _prep_inputs(inputs)
    trace = bool(int(os.environ.get("KERNEL_TRACE", "0")))
    res = run_bass_kernel_spmd(nc, in_maps, core_ids=list(range(NCORES)),
                               trace=trace)
    if trace:
        _CACHE["last_result"] = res
        print(f"HW exec time: {res.exec_time_ns} ns", flush=True)

    out = np.concatenate([res.results[c]["out_sl"] for c in range(NCORES)], axis=0)
    return out.reshape(B, S, D)
